# revision 1
# baseline (speedup 1.0000x reference)
"""GCN (3-layer GCNConv + global_add_pool + linear head) on 8 Trainium2 cores.

Strategy:
 - Nodes sharded across 8 cores on graph-id-aligned boundaries (pooling local).
 - Edges partitioned by dst owner. Per core, edges ordered chunk-major
   (src_row % 4 -> int16 gather index fits), then by 128-node dst window,
   padded per (chunk, window) group to multiples of 128 and uniform tile
   counts across cores (single SPMD program).
 - Per layer: dense z = h @ W on PE (transpose-on-the-fly), zn = z * dinv,
   AllGather zn -> full table in DRAM, dma_gather 256B rows per edge,
   segment-sum via one-hot matmul into PSUM per window, accumulated in SBUF
   across the 4 chunk passes, then tanh(dinv*acc + b) in place.
 - Pooling: one-hot(graph id) matmul into a [64, 512] PSUM tile; final
   linear head + tanh on device.
"""

import hashlib
import sys

for _p in ("/opt/trn_rl_repo",):
    if _p not in sys.path:
        sys.path.insert(0, _p)

import numpy as np

P = 128
WIN = 128          # dst-window width (nodes)
NCHUNK = 4         # src chunks (int16 index limit: 8S/4 <= 32767)
GRANULE = 4096     # idxs per dma_gather instruction (ring: 3 in flight)
N_CORES = 8
N_GRAPHS = 2048    # problem constant
GMAX = 512         # per-core graph-count upper bound (psum free dim)


# ----------------------------------------------------------------------------
# Host-side sharding / edge bucketing (index manipulation only, no float math)
# ----------------------------------------------------------------------------

def _preprocess(edge_index, batch, n_nodes, n_graphs):
    C = N_CORES
    src = np.asarray(edge_index[0], dtype=np.int64)
    dst = np.asarray(edge_index[1], dtype=np.int64)
    batch = np.asarray(batch, dtype=np.int64)
    N = n_nodes

    # graph-aligned node shard boundaries
    gstart = np.searchsorted(batch, np.arange(n_graphs + 1))  # [G+1], gstart[G] = N
    node_bnds = [0]
    g_bnds = [0]
    for c in range(1, C):
        tgt = (c * N) // C
        g = int(np.searchsorted(gstart, tgt))
        # candidates g-1, g: pick nearest boundary node
        if g > 0 and abs(int(gstart[g - 1]) - tgt) <= abs(int(gstart[min(g, n_graphs)]) - tgt):
            g = g - 1
        g = min(max(g, g_bnds[-1]), n_graphs)
        g_bnds.append(g)
        node_bnds.append(int(gstart[g]))
    node_bnds.append(N)
    g_bnds.append(n_graphs)
    node_bnds = np.array(node_bnds, dtype=np.int64)          # [C+1]
    g_bnds = np.array(g_bnds, dtype=np.int64)                # [C+1]
    node_cnt = node_bnds[1:] - node_bnds[:-1]
    g_cnt = g_bnds[1:] - g_bnds[:-1]
    assert g_cnt.max() < GMAX - 1, g_cnt

    S = int(-(-node_cnt.max() // P) * P)                     # padded shard size
    NW = S // WIN                                            # windows per core
    assert 2 * S <= 32767, S                                 # int16 gather idx bound

    owner = np.searchsorted(node_bnds[1:], np.arange(N), side="right")
    local = np.arange(N) - node_bnds[owner]
    row = owner * S + local                                  # table row per node

    deg = np.bincount(dst, minlength=N).astype(np.float32) + 1.0

    # edge stream (+ self loops)
    e_src = np.concatenate([src, np.arange(N)])
    e_dst = np.concatenate([dst, np.arange(N)])
    e_owner = owner[e_dst]
    e_dl = local[e_dst]
    e_row = row[e_src]
    e_chunk = (e_row & (NCHUNK - 1)).astype(np.int64)
    e_idx = (e_row >> 2).astype(np.int16)
    e_win = e_dl >> 7

    key = (e_chunk * NW + e_win) * C + e_owner               # chunk-major, then window
    order = np.argsort(key, kind="stable")
    cnt = np.bincount(key, minlength=NCHUNK * NW * C).reshape(NCHUNK, NW, C)

    tiles_kw = -(-cnt.max(axis=2) // P)                      # [NCHUNK, NW] uniform tiles
    # chunk-0 pass initializes the SBUF accumulator (copy): force >=1 tile/window
    tiles_kw[0] = np.maximum(tiles_kw[0], 1)
    pad_kw = tiles_kw * P                                    # padded group sizes
    E_PAD = int(pad_kw.sum())
    # group start offsets in the uniform stream (same for all cores)
    goff = np.zeros((NCHUNK, NW), dtype=np.int64)
    goff.flat[1:] = np.cumsum(pad_kw.flat)[:-1]

    idx16 = np.zeros((C, E_PAD), dtype=np.int16)             # pad -> idx 0 (valid row)
    dstl = np.full((C, E_PAD), -1.0, dtype=np.float32)       # pad -> -1 (one-hot miss)

    # place real edges
    so = order
    r_owner = e_owner[so]
    r_chunk = e_chunk[so]
    r_win = e_win[so]
    # position within (chunk, win, owner) group = running index
    rkey = (r_chunk * NW + r_win) * C + r_owner
    # stable sort => positions are 0..cnt-1 in order of appearance
    pos = np.zeros(len(so), dtype=np.int64)
    _, first_idx, inv = np.unique(rkey, return_index=True, return_inverse=True)
    pos = np.arange(len(so)) - first_idx[inv]
    slot = goff[r_chunk, r_win] + pos
    idx16[r_owner, slot] = e_idx[so]
    dstl[r_owner, slot] = (e_dl[so] - r_win * WIN).astype(np.float32)

    # per-tile metadata (uniform): window id, group-first, group-last
    TILES = E_PAD // P
    tile_win = np.zeros(TILES, dtype=np.int64)
    tile_first = np.zeros(TILES, dtype=bool)
    tile_last = np.zeros(TILES, dtype=bool)
    for k in range(NCHUNK):
        for w in range(NW):
            t0 = goff[k, w] // P
            nt = int(tiles_kw[k, w])
            if nt == 0:
                continue
            tile_win[t0:t0 + nt] = w
            tile_first[t0] = True
            tile_last[t0 + nt - 1] = True
    # chunk segment boundaries (in idx positions)
    chunk_off = [int(goff[k, 0]) for k in range(NCHUNK)] + [E_PAD]

    # gather-layout idx: [16, E_PAD/16] with [p, s] = stream[s*16+p]
    idx_wrapped = np.ascontiguousarray(
        idx16.reshape(C, E_PAD // 16, 16).transpose(0, 2, 1))
    # dstl layout [128, E_PAD/128] with [p, t] = stream[t*128+p]
    dstl_wrapped = np.ascontiguousarray(
        dstl.reshape(C, TILES, P).transpose(0, 2, 1))

    # per-core padded node arrays
    deg_pad = np.ones((C, S), dtype=np.float32)
    batchl = np.full((C, S), float(GMAX - 1), dtype=np.float32)
    for c in range(C):
        n0, n1 = int(node_bnds[c]), int(node_bnds[c + 1])
        deg_pad[c, : n1 - n0] = deg[n0:n1]
        batchl[c, : n1 - n0] = (batch[n0:n1] - g_bnds[c]).astype(np.float32)
    batchl_wrapped = np.ascontiguousarray(
        batchl.reshape(C, NW, P).transpose(0, 2, 1))         # [C, 128, NW]

    return dict(
        S=S, NW=NW, E_PAD=E_PAD, TILES=TILES,
        node_bnds=node_bnds, g_bnds=g_bnds, node_cnt=node_cnt, g_cnt=g_cnt,
        idx_wrapped=idx_wrapped, dstl_wrapped=dstl_wrapped,
        batchl_wrapped=batchl_wrapped, deg_pad=deg_pad,
        tile_win=tile_win, tile_first=tile_first, tile_last=tile_last,
        chunk_off=chunk_off,
    )


# ----------------------------------------------------------------------------
# Bass program builder
# ----------------------------------------------------------------------------

def _build_program(meta, d_in, h_dim, n_cls):
    import concourse.bacc as bacc
    import concourse.mybir as mybir
    import concourse.tile as tile
    from concourse import library_config

    S, NW, E_PAD = meta["S"], meta["NW"], meta["E_PAD"]
    tile_win = meta["tile_win"]
    tile_first = meta["tile_first"]
    tile_last = meta["tile_last"]
    chunk_off = meta["chunk_off"]
    f32 = mybir.dt.float32
    AOT = mybir.ActivationFunctionType
    ALU = mybir.AluOpType

    nc = bacc.Bacc("TRN2", target_bir_lowering=False, debug=False,
                   num_devices=N_CORES)

    # --- I/O ---
    x_d = nc.dram_tensor("x_loc", [S, d_in], f32, kind="ExternalInput").ap()
    deg_d = nc.dram_tensor("deg_loc", [S], f32, kind="ExternalInput").ap()
    idx_d = nc.dram_tensor("idx16", [P, E_PAD // 16], mybir.dt.int16,
                           kind="ExternalInput").ap()
    dstl_d = nc.dram_tensor("dstl", [P, E_PAD // P], f32,
                            kind="ExternalInput").ap()
    batchl_d = nc.dram_tensor("batchl", [P, NW], f32, kind="ExternalInput").ap()
    W_d = [nc.dram_tensor("W1", [d_in, h_dim], f32, kind="ExternalInput").ap(),
           nc.dram_tensor("W2", [h_dim, h_dim], f32, kind="ExternalInput").ap(),
           nc.dram_tensor("W3", [h_dim, h_dim], f32, kind="ExternalInput").ap()]
    Wf_d = nc.dram_tensor("Wf", [h_dim, n_cls], f32, kind="ExternalInput").ap()
    b_d = [nc.dram_tensor(f"b{i+1}b", [P, h_dim], f32, kind="ExternalInput").ap()
           for i in range(3)]
    bf_d = nc.dram_tensor("bfb", [P, n_cls], f32, kind="ExternalInput").ap()
    out_d = nc.dram_tensor("out", [GMAX, n_cls], f32, kind="ExternalOutput").ap()
    ident_d = nc.dram_tensor("ident", [P, P], f32, kind="ExternalInput").ap()
    iota_w_d = nc.dram_tensor("iota_w", [P, WIN], f32, kind="ExternalInput").ap()
    iota_g_d = nc.dram_tensor("iota_g", [P, GMAX], f32, kind="ExternalInput").ap()

    zn_d = nc.dram_tensor("zn_loc", [S, h_dim], f32).ap()
    table_d = nc.dram_tensor("table", [N_CORES * S, h_dim], f32,
                             addr_space="Shared").ap()
    chunk_views = table_d.rearrange("(n four) d -> four n d", four=NCHUNK)
    rg = [list(range(N_CORES))]

    with tile.TileContext(nc) as tc:
        with (
            tc.tile_pool(name="persist", bufs=1) as pp,
            tc.tile_pool(name="msg", bufs=4) as msgp,
            tc.tile_pool(name="work", bufs=4) as wp,
            tc.tile_pool(name="dense", bufs=3) as dp,
            tc.tile_pool(name="psum", bufs=2, space="PSUM") as psp,
            tc.tile_pool(name="psum1", bufs=2, space="PSUM") as ps1,
            tc.tile_pool(name="pool_ps", bufs=1, space="PSUM") as poolps,
        ):
            # --- persistent tiles ---
            nc.gpsimd.load_library(library_config.mlp)
            ident = pp.tile([P, P], f32, tag="ident")
            nc.sync.dma_start(ident[:], ident_d[:])
            iota_w = pp.tile([P, WIN], f32, tag="iota_w")
            nc.sync.dma_start(iota_w[:], iota_w_d[:])
            iota_g = pp.tile([P, GMAX], f32, tag="iota_g")
            nc.sync.dma_start(iota_g[:], iota_g_d[:])

            W_sb = []
            for i in range(3):
                k = d_in if i == 0 else h_dim
                t = pp.tile([k, h_dim], f32, tag=f"W{i}")
                nc.sync.dma_start(t[:], W_d[i][:])
                W_sb.append(t)
            Wf_sb = pp.tile([h_dim, n_cls], f32, tag="Wf")
            nc.sync.dma_start(Wf_sb[:], Wf_d[:])
            b_sb = []
            for i in range(3):
                t = pp.tile([P, h_dim], f32, tag=f"b{i}")
                nc.sync.dma_start(t[:], b_d[i][:])
                b_sb.append(t)
            bf_sb = pp.tile([P, n_cls], f32, tag="bf")
            nc.sync.dma_start(bf_sb[:], bf_d[:])

            idx_sb = pp.tile([P, E_PAD // 16], mybir.dt.int16, tag="idx")
            nc.sync.dma_start(idx_sb[:], idx_d[:])
            dstl_sb = pp.tile([P, E_PAD // P], f32, tag="dstl")
            nc.sync.dma_start(dstl_sb[:], dstl_d[:])
            batchl_sb = pp.tile([P, NW], f32, tag="batchl")
            nc.sync.dma_start(batchl_sb[:], batchl_d[:])

            dinv = pp.tile([P, NW], f32, tag="dinv")
            deg_col = pp.tile([P, NW], f32, tag="degc")
            nc.sync.dma_start(deg_col[:], deg_d.rearrange("(t p) -> p t", p=P))
            # dinv = 1/sqrt(deg): sqrt on ACT, then DVE reciprocal
            nc.scalar.activation(deg_col[:], deg_col[:], AOT.Sqrt)
            nc.vector.reciprocal(dinv[:], deg_col[:])

            bufA = pp.tile([P, NW * h_dim], f32, tag="bufA")

            # === 3 GCN layers ===
            for layer in range(3):
                # ---- dense: zn = (h_in @ W) * dinv, tile by tile ----
                for t in range(NW):
                    if layer == 0:
                        xt = dp.tile([P, d_in], f32, tag="xt")
                        nc.sync.dma_start(xt[:], x_d[t * P:(t + 1) * P, :])
                        tp = ps1.tile([d_in, P], f32, tag="tps")
                        nc.tensor.transpose(tp[:], xt[:], ident[:])
                        sbT = dp.tile([d_in, P], f32, tag="sbT")
                        nc.vector.tensor_copy(sbT[:], tp[:])
                        kdim = d_in
                    else:
                        tp = ps1.tile([h_dim, P], f32, tag="tps")
                        nc.tensor.transpose(
                            tp[:], bufA[:, t * h_dim:(t + 1) * h_dim], ident[:])
                        sbT = dp.tile([h_dim, P], f32, tag="sbT")
                        nc.vector.tensor_copy(sbT[:], tp[:])
                        kdim = h_dim
                    zps = ps1.tile([P, h_dim], f32, tag="zps")
                    nc.tensor.matmul(zps[:], lhsT=sbT[:], rhs=W_sb[layer][:],
                                     start=True, stop=True)
                    nc.vector.tensor_scalar(
                        out=bufA[:, t * h_dim:(t + 1) * h_dim], in0=zps[:],
                        scalar1=dinv[:, t:t + 1], scalar2=None, op0=ALU.mult)

                # ---- publish zn + AllGather ----
                nc.sync.dma_start(
                    zn_d.rearrange("(t p) d -> p t d", p=P),
                    bufA[:].rearrange("p (t d) -> p t d", d=h_dim))
                nc.gpsimd.collective_compute(
                    "AllGather", ALU.bypass, replica_groups=rg,
                    ins=[zn_d[:]], outs=[table_d[:]])

                # ---- sparse aggregation: chunk-major gather + one-hot matmul ----
                wpsum = None
                for k in range(NCHUNK):
                    seg0, seg1 = chunk_off[k], chunk_off[k + 1]
                    for a in range(seg0, seg1, GRANULE):
                        gsz = min(GRANULE, seg1 - a)
                        gT = gsz // P
                        msg = msgp.tile([P, gT * h_dim], f32, tag="msg")
                        nc.gpsimd.dma_gather(
                            msg[:].rearrange("p (t d) -> p t d", d=h_dim),
                            chunk_views[k],
                            idx_sb[:, a // 16:(a + gsz) // 16],
                            gsz, gsz, h_dim, elem_step=NCHUNK * h_dim,
                            single_packet=False)
                        for i in range(gT):
                            t = a // P + i
                            w = int(tile_win[t])
                            oh = wp.tile([P, WIN], f32, tag="oh")
                            nc.vector.tensor_scalar(
                                out=oh[:], in0=iota_w[:],
                                scalar1=dstl_sb[:, t:t + 1], scalar2=None,
                                op0=ALU.is_equal)
                            if tile_first[t]:
                                wpsum = psp.tile([WIN, h_dim], f32, tag="wps")
                            nc.tensor.matmul(
                                wpsum[:], lhsT=oh[:],
                                rhs=msg[:, i * h_dim:(i + 1) * h_dim],
                                start=bool(tile_first[t]),
                                stop=bool(tile_last[t]))
                            if tile_last[t]:
                                dst = bufA[:, w * h_dim:(w + 1) * h_dim]
                                if k == 0:
                                    nc.vector.tensor_copy(dst, wpsum[:])
                                else:
                                    nc.vector.tensor_tensor(
                                        out=dst, in0=dst, in1=wpsum[:],
                                        op=ALU.add)

                # ---- flush: h = tanh(dinv * acc + b), in place ----
                for w in range(NW):
                    sl = bufA[:, w * h_dim:(w + 1) * h_dim]
                    tmp = wp.tile([P, h_dim], f32, tag="ftmp")
                    nc.vector.tensor_scalar(
                        out=tmp[:], in0=sl, scalar1=dinv[:, w:w + 1],
                        scalar2=None, op0=ALU.mult)
                    nc.vector.tensor_tensor(out=tmp[:], in0=tmp[:],
                                            in1=b_sb[layer][:], op=ALU.add)
                    nc.scalar.activation(sl, tmp[:], AOT.Tanh)

            # === pooling: pooledT[64, GMAX] = sum_h3 by graph ===
            poolT = poolps.tile([h_dim, GMAX], f32, tag="poolT")
            for t in range(NW):
                ohg = wp.tile([P, GMAX], f32, tag="ohg")
                nc.vector.tensor_scalar(
                    out=ohg[:], in0=iota_g[:], scalar1=batchl_sb[:, t:t + 1],
                    scalar2=None, op0=ALU.is_equal)
                nc.tensor.matmul(poolT[:],
                                 lhsT=bufA[:, t * h_dim:(t + 1) * h_dim],
                                 rhs=ohg[:], start=(t == 0), stop=(t == NW - 1))
            poolS = pp.tile([h_dim, GMAX], f32, tag="poolS")
            nc.vector.tensor_copy(poolS[:], poolT[:])

            # === head: out = tanh(pooled @ Wf + bf) ===
            for gt in range(GMAX // P):
                fps = psp.tile([P, n_cls], f32, tag="wps")
                nc.tensor.matmul(fps[:], lhsT=poolS[:, gt * P:(gt + 1) * P],
                                 rhs=Wf_sb[:], start=True, stop=True)
                ot = wp.tile([P, n_cls], f32, tag="ot")
                nc.vector.tensor_tensor(out=ot[:], in0=fps[:], in1=bf_sb[:],
                                        op=ALU.add)
                nc.scalar.activation(ot[:], ot[:], AOT.Tanh)
                nc.sync.dma_start(out_d[gt * P:(gt + 1) * P, :], ot[:])

    nc.compile()
    return nc


# ----------------------------------------------------------------------------
# Runner (persistent compiled program + per-core inputs)
# ----------------------------------------------------------------------------

class Runner:
    def __init__(self, meta, nc, d_in, h_dim, n_cls):
        self.meta = meta
        self.nc = nc
        self.d_in, self.h_dim, self.n_cls = d_in, h_dim, n_cls

    def in_maps(self, x, W1, b1, W2, b2, W3, b3, Wf, bf):
        m = self.meta
        S = m["S"]
        C = N_CORES
        x = np.asarray(x, np.float32)
        maps = []
        reps = dict(
            W1=np.asarray(W1, np.float32), W2=np.asarray(W2, np.float32),
            W3=np.asarray(W3, np.float32), Wf=np.asarray(Wf, np.float32),
            b1b=np.broadcast_to(np.asarray(b1, np.float32), (P, self.h_dim)).copy(),
            b2b=np.broadcast_to(np.asarray(b2, np.float32), (P, self.h_dim)).copy(),
            b3b=np.broadcast_to(np.asarray(b3, np.float32), (P, self.h_dim)).copy(),
            bfb=np.broadcast_to(np.asarray(bf, np.float32), (P, self.n_cls)).copy(),
            ident=np.eye(P, dtype=np.float32),
            iota_w=np.broadcast_to(np.arange(WIN, dtype=np.float32), (P, WIN)).copy(),
            iota_g=np.broadcast_to(np.arange(GMAX, dtype=np.float32), (P, GMAX)).copy(),
        )
        for c in range(C):
            n0, n1 = int(m["node_bnds"][c]), int(m["node_bnds"][c + 1])
            xl = np.zeros((S, self.d_in), np.float32)
            xl[: n1 - n0] = x[n0:n1]
            maps.append(dict(
                x_loc=xl,
                deg_loc=m["deg_pad"][c],
                idx16=np.tile(m["idx_wrapped"][c], (8, 1)),
                dstl=m["dstl_wrapped"][c],
                batchl=m["batchl_wrapped"][c],
                **reps,
            ))
        return maps

    def run(self, maps):
        from concourse.bass_utils import run_bass_kernel_spmd
        res = run_bass_kernel_spmd(self.nc, maps, list(range(N_CORES)))
        return self.assemble(res.results)

    def make_timed(self, maps):
        """Build a callable with inputs resident on device; each call runs the
        NEFF once and returns per-core outputs. For timing (transfer excluded)."""
        import jax
        import concourse.mybir as mybir
        from concourse import bass2jax
        from jax.experimental.shard_map import shard_map
        from jax.sharding import Mesh, NamedSharding, PartitionSpec

        nc = self.nc
        bass2jax.install_neuronx_cc_hook()
        partition_name = (nc.partition_id_tensor.name
                          if nc.partition_id_tensor else None)
        in_names, out_names, out_avals, zero_outs = [], [], [], []
        for alloc in nc.m.functions[0].allocations:
            if not isinstance(alloc, mybir.MemoryLocationSet):
                continue
            name = alloc.memorylocations[0].name
            if alloc.kind == "ExternalInput":
                if name != partition_name:
                    in_names.append(name)
            elif alloc.kind == "ExternalOutput":
                shape = tuple(alloc.tensor_shape)
                dtype = mybir.dt.np(alloc.dtype)
                out_names.append(name)
                out_avals.append(jax.core.ShapedArray(shape, dtype))
                zero_outs.append(np.zeros(shape, dtype))
        n_params = len(in_names)
        all_in = list(in_names) + list(out_names)
        if partition_name is not None:
            all_in.append(partition_name)
        donate = tuple(range(n_params, n_params + len(out_names)))

        def _body(*args):
            operands = list(args)
            if partition_name is not None:
                operands.append(bass2jax.partition_id_tensor())
            return tuple(bass2jax._bass_exec_p.bind(
                *operands, out_avals=tuple(out_avals), in_names=tuple(all_in),
                out_names=tuple(out_names), lowering_input_output_aliases=(),
                sim_require_finite=True, sim_require_nnan=True, nc=nc))

        devices = jax.devices()[:N_CORES]
        mesh = Mesh(np.asarray(devices), ("core",))
        spec = NamedSharding(mesh, PartitionSpec("core"))
        fn = jax.jit(shard_map(_body, mesh=mesh,
                               in_specs=(PartitionSpec("core"),) * (n_params + len(out_names)),
                               out_specs=(PartitionSpec("core"),) * len(out_names)),
                     donate_argnums=donate, keep_unused=True)
        dev_in = [jax.device_put(
            np.concatenate([np.asarray(maps[c][nm]) for c in range(N_CORES)], axis=0),
            spec) for nm in in_names]
        zshapes = [(N_CORES * z.shape[0], *z.shape[1:]) for z in zero_outs]
        zdtypes = [z.dtype for z in zero_outs]

        def call():
            zs = [jax.device_put(np.zeros(s, d), spec)
                  for s, d in zip(zshapes, zdtypes)]
            outs = fn(*dev_in, *zs)
            return [o.block_until_ready() for o in outs]

        return call, out_names, out_avals

    def assemble(self, results):
        m = self.meta
        outs = []
        for c in range(N_CORES):
            outs.append(results[c]["out"][: int(m["g_cnt"][c])])
        return np.concatenate(outs, axis=0)


_CACHE = {}


def _get_runner(edge_index, batch, n_nodes, n_graphs, d_in, h_dim, n_cls):
    key = (hashlib.sha1(np.ascontiguousarray(edge_index).tobytes()).hexdigest(),
           hashlib.sha1(np.ascontiguousarray(batch).tobytes()).hexdigest(),
           n_nodes, n_graphs, d_in, h_dim, n_cls)
    r = _CACHE.get(key)
    if r is None:
        meta = _preprocess(edge_index, batch, n_nodes, n_graphs)
        nc = _build_program(meta, d_in, h_dim, n_cls)
        r = Runner(meta, nc, d_in, h_dim, n_cls)
        _CACHE[key] = r
    return r


def kernel(x, edge_index, batch, W1, b1, W2, b2, W3, b3, Wf, bf):
    x = np.asarray(x)
    r = _get_runner(np.asarray(edge_index), np.asarray(batch), x.shape[0],
                    N_GRAPHS, x.shape[1], np.asarray(W1).shape[1],
                    np.asarray(Wf).shape[1])
    maps = r.in_maps(x, W1, b1, W2, b2, W3, b3, Wf, bf)
    return r.run(maps)



# revision 2
# speedup vs baseline: 1.0387x; 1.0387x over previous
"""GCN (3-layer GCNConv + global_add_pool + linear head) on 8 Trainium2 cores, v2.

Structure (per core, SPMD single program):
 - Nodes sharded across 8 cores on graph-id-aligned boundaries; S padded to
   a multiple of 512 -> NW windows of 128 nodes, 4 quarters of NWQ windows.
 - Global bf16 table [8*S, 128] (each row = node's 64 zn features duplicated
   to fill the 256B gather-row minimum), split into 4 window-aligned quarter
   tensors for AllGather pipelining.
 - Edges (no self loops; self contribution folded into the flush) partitioned
   by dst owner, grouped by (src quarter = pass, dst window); per-group tile
   counts maxed across cores (uniform SPMD program), pads masked by zero
   one-hot columns.
 - One-hot scatter matrices precomputed on HOST (static edge structure),
   streamed from DRAM via HWDGE per granule: no DVE one-hot work at all.
 - Per granule (<=32 tiles = 4096 edges): gpsimd.dma_gather 256B rows from
   the quarter table; per tile one bf16 matmul (lhsT=onehot, rhs=msg) into a
   per-(pass,window) PSUM chain; closed windows accumulate into SBUF bufA.
 - flush(w): h = tanh(dinv*(acc + zn) + b) (DVE+ACT); dense(w) for the next
   layer: PE transpose + matmul + dinv scale; publish + AllGather per quarter
   as soon as its windows are done -> collectives hide under the gather
   stream of the remaining windows.
 - Pooling rides the last layer's flush: one-hot(graph) matmul into a
   [64, 512] PSUM tile; final linear head + tanh on device.
"""

import hashlib
import sys

for _p in ("/opt/trn_rl_repo",):
    if _p not in sys.path:
        sys.path.insert(0, _p)

import numpy as np

P = 128
H = 64
NQ = 4             # passes / table quarters (int16 idx + AG pipelining)
GRT = 32           # tiles per gather granule (4096 idxs; SWDGE ring limit)
N_CORES = 8
N_GRAPHS = 2048
GMAX = 512         # per-core graph-count upper bound (psum free dim)


# ----------------------------------------------------------------------------
# Host-side sharding / edge bucketing / one-hot construction
# ----------------------------------------------------------------------------

def _preprocess(edge_index, batch, n_nodes, n_graphs, s_extra=0):
    import ml_dtypes
    bf16 = ml_dtypes.bfloat16
    C = N_CORES
    src = np.asarray(edge_index[0], dtype=np.int64)
    dst = np.asarray(edge_index[1], dtype=np.int64)
    batch = np.asarray(batch, dtype=np.int64)
    N = n_nodes

    # graph-aligned node shard boundaries (balanced by node count)
    gstart = np.searchsorted(batch, np.arange(n_graphs + 1))
    node_bnds = [0]
    g_bnds = [0]
    for c in range(1, C):
        tgt = (c * N) // C
        g = int(np.searchsorted(gstart, tgt))
        if g > 0 and abs(int(gstart[g - 1]) - tgt) <= abs(int(gstart[min(g, n_graphs)]) - tgt):
            g = g - 1
        g = min(max(g, g_bnds[-1]), n_graphs)
        g_bnds.append(g)
        node_bnds.append(int(gstart[g]))
    node_bnds.append(N)
    g_bnds.append(n_graphs)
    node_bnds = np.array(node_bnds, dtype=np.int64)
    g_bnds = np.array(g_bnds, dtype=np.int64)
    node_cnt = node_bnds[1:] - node_bnds[:-1]
    g_cnt = g_bnds[1:] - g_bnds[:-1]
    assert g_cnt.max() < GMAX - 1, g_cnt

    S = int(-(-node_cnt.max() // 512) * 512) + s_extra
    NW = S // P                      # windows per core
    NWQ = NW // NQ                   # windows per quarter
    SQ = S // NQ                     # nodes per quarter per core
    QROWS = C * SQ                   # rows per quarter table
    assert QROWS <= 32768

    owner = np.searchsorted(node_bnds[1:], np.arange(N), side="right")
    old_local = np.arange(N) - node_bnds[owner]

    deg = np.bincount(dst, minlength=N).astype(np.float32) + 1.0

    # --- node placement: quarter snake by in-degree, then 4-dim balanced
    # window packing so per-(pass,window) counts stay under tile multiples ---
    indeg = np.bincount(dst, minlength=N)
    quarter_of = np.zeros(N, dtype=np.int64)     # per global node
    perm = [None] * C                            # new local -> old local
    for c in range(C):
        n0, n1 = int(node_bnds[c]), int(node_bnds[c + 1])
        ids = np.arange(n0, n1)
        order_d = ids[np.argsort(-indeg[n0:n1], kind="stable")]
        # snake-deal across quarters for near-equal quarter in-degree
        qs = np.tile(np.concatenate([np.arange(NQ), np.arange(NQ)[::-1]]),
                     -(-len(ids) // (2 * NQ)))[:len(ids)]
        # capacity: SQ nodes per quarter
        qcnt = np.zeros(NQ, dtype=np.int64)
        qa = np.empty(len(ids), dtype=np.int64)
        for i, nd in enumerate(order_d):
            q = qs[i]
            if qcnt[q] >= SQ:
                q = int(np.argmin(qcnt))
            qa[i] = q
            qcnt[q] += 1
        quarter_of[order_d] = qa
    # in-degree vector by src quarter (fixed now for all nodes)
    e_core = owner[dst]
    e_pass_g = quarter_of[src]
    vdim = np.zeros((N, NQ), dtype=np.int64)
    np.add.at(vdim, (dst, e_pass_g), 1)
    for c in range(C):
        n0, n1 = int(node_bnds[c]), int(node_bnds[c + 1])
        new_local = np.empty(n1 - n0, dtype=np.int64)
        pos = 0
        for q in range(NQ):
            ids = np.arange(n0, n1)[quarter_of[n0:n1] == q]
            # greedy 4-dim balanced packing into NWQ windows of 128 nodes
            o = np.argsort(-vdim[ids].sum(axis=1), kind="stable")
            ids = ids[o]
            loads = np.zeros((NWQ, NQ), dtype=np.int64)
            wcnt = np.zeros(NWQ, dtype=np.int64)
            wslot = np.empty(len(ids), dtype=np.int64)
            for i, nd in enumerate(ids):
                cand = np.max(loads + vdim[nd], axis=1) + (wcnt >= P) * (1 << 30)
                wbin = int(np.argmin(cand))
                wslot[i] = wbin
                loads[wbin] += vdim[nd]
                wcnt[wbin] += 1
            # new local position: quarter base + window*128 + intra slot
            iorder = np.lexsort((np.arange(len(ids)), wslot))
            intra = np.zeros(len(ids), dtype=np.int64)
            woff = np.zeros(NWQ, dtype=np.int64)
            for i in iorder:
                wbin = wslot[i]
                intra[i] = woff[wbin]
                woff[wbin] += 1
            new_local[ids - n0] = q * SQ + wslot * P + intra
            pos += len(ids)
        perm_c = np.full(S, -1, dtype=np.int64)   # new local -> old local
        perm_c[new_local] = np.arange(n1 - n0)
        # compact: pad slots (-1) stay; used by in_maps for x/deg/batch perm
        perm[c] = perm_c
    local = np.empty(N, dtype=np.int64)
    for c in range(C):
        n0, n1 = int(node_bnds[c]), int(node_bnds[c + 1])
        inv = np.empty(n1 - n0, dtype=np.int64)
        mask = perm[c] >= 0
        inv[perm[c][mask]] = np.nonzero(mask)[0]
        local[n0:n1] = inv

    # edge bucketing: core = owner[dst]; pass = quarter(src) (== local//SQ)
    e_dl = local[dst]                       # dst local id [0, S)
    s_local = local[src]
    e_pass = s_local // SQ                  # src quarter = pass
    assert (e_pass == e_pass_g).all()
    e_qrow = owner[src] * SQ + (s_local - e_pass * SQ)   # row within quarter
    e_win = e_dl // P

    # group key (pass, window); counts per core
    gkey = (e_pass * NW + e_win)
    NGRP = NQ * NW
    cnt = np.zeros((C, NGRP), dtype=np.int64)
    np.add.at(cnt, (e_core, gkey), 1)
    tiles_g = -(-cnt.max(axis=0) // P)      # uniform tiles per group
    if tiles_g.max() > 4 and s_extra == 0:
        # packing spilled past the 4-tile quantum; retry with more slack
        return _preprocess(edge_index, batch, n_nodes, n_graphs, s_extra=512)
    TILES = int(tiles_g.sum())
    E_PAD = TILES * P

    goff_t = np.zeros(NGRP, dtype=np.int64)          # tile offset per group
    goff_t[1:] = np.cumsum(tiles_g)[:-1]
    goff_s = goff_t * P                               # slot offset

    # place real edges: slot = group offset + running index per (core, group)
    order = np.lexsort((e_dl, gkey, e_core))          # by core, group, dl
    r_core = e_core[order]
    r_key = gkey[order]
    ckey = r_core * NGRP + r_key
    uniq, first_idx, inv = np.unique(ckey, return_index=True, return_inverse=True)
    pos = np.arange(len(order)) - first_idx[inv]
    slot = goff_s[r_key] + pos

    idx16 = np.zeros((C, E_PAD), dtype=np.int16)
    dstc = np.full((C, E_PAD), -1, dtype=np.int64)    # one-hot col; -1 = pad
    idx16[r_core, slot] = e_qrow[order].astype(np.int16)
    dstc[r_core, slot] = (e_dl[order] - (r_key % NW) * P)

    # per-tile metadata
    tile_win = np.zeros(TILES, dtype=np.int64)
    tile_pass = np.zeros(TILES, dtype=np.int64)
    tile_first = np.zeros(TILES, dtype=bool)
    tile_last = np.zeros(TILES, dtype=bool)
    for gi in range(NGRP):
        t0, nt = int(goff_t[gi]), int(tiles_g[gi])
        if nt == 0:
            continue
        q, w = gi // NW, gi % NW
        tile_win[t0:t0 + nt] = w
        tile_pass[t0:t0 + nt] = q
        tile_first[t0] = True
        tile_last[t0 + nt - 1] = True

    pass_t = [int(tiles_g[q * NW:(q + 1) * NW].sum()) for q in range(NQ)]
    pass_t0 = np.zeros(NQ + 1, dtype=np.int64)
    pass_t0[1:] = np.cumsum(pass_t)

    # per-window first/last contributing pass
    has_qw = (tiles_g.reshape(NQ, NW) > 0)
    first_pass = np.where(has_qw.any(axis=0), has_qw.argmax(axis=0), -1)
    last_pass = np.where(has_qw.any(axis=0),
                         NQ - 1 - has_qw[::-1].argmax(axis=0), -1)

    # one-hot tiles, bf16 [C, 128, TILES*128]: oh[c, p, t*128+col]
    oh = np.zeros((C, P, TILES, P), dtype=np.uint8)
    pp = slot % P
    tt = slot // P
    cc = dstc[r_core, slot]
    m = cc >= 0
    oh[r_core[m], pp[m], tt[m], cc[m]] = 1
    ohb = oh.reshape(C, P, TILES * P).astype(bf16)

    # idx gather layout [16, E_PAD/16] -> replicate to 128 partitions
    idx_wrapped = np.ascontiguousarray(
        idx16.reshape(C, E_PAD // 16, 16).transpose(0, 2, 1))

    # per-core padded node arrays (permuted layout)
    deg_pad = np.ones((C, S), dtype=np.float32)
    batchl = np.full((C, S), float(GMAX - 1), dtype=np.float32)
    for c in range(C):
        n0, n1 = int(node_bnds[c]), int(node_bnds[c + 1])
        mask = perm[c] >= 0
        deg_pad[c, mask] = deg[n0:n1][perm[c][mask]]
        batchl[c, mask] = (batch[n0:n1][perm[c][mask]] - g_bnds[c]).astype(
            np.float32)
    deg_w = np.ascontiguousarray(deg_pad.reshape(C, NW, P).transpose(0, 2, 1))
    batchl_w = np.ascontiguousarray(batchl.reshape(C, NW, P).transpose(0, 2, 1))

    return dict(
        S=S, NW=NW, NWQ=NWQ, SQ=SQ, QROWS=QROWS, TILES=TILES, E_PAD=E_PAD,
        node_bnds=node_bnds, g_bnds=g_bnds, node_cnt=node_cnt, g_cnt=g_cnt,
        idx_wrapped=idx_wrapped, ohb=ohb, deg_w=deg_w, batchl_w=batchl_w,
        perm=perm,
        tile_win=tile_win, tile_pass=tile_pass,
        tile_first=tile_first, tile_last=tile_last,
        pass_t0=pass_t0, first_pass=first_pass, last_pass=last_pass,
    )


# ----------------------------------------------------------------------------
# Bass program builder
# ----------------------------------------------------------------------------

def _build_program(meta, d_in, h_dim, n_cls):
    import concourse.bacc as bacc
    import concourse.mybir as mybir
    import concourse.tile as tile

    S, NW, NWQ, SQ = meta["S"], meta["NW"], meta["NWQ"], meta["SQ"]
    QROWS, TILES = meta["QROWS"], meta["TILES"]
    tile_win = meta["tile_win"]
    tile_first = meta["tile_first"]
    tile_last = meta["tile_last"]
    pass_t0 = meta["pass_t0"]
    first_pass = meta["first_pass"]
    last_pass = meta["last_pass"]
    f32 = mybir.dt.float32
    bf16 = mybir.dt.bfloat16
    i16 = mybir.dt.int16
    AOT = mybir.ActivationFunctionType
    ALU = mybir.AluOpType

    nc = bacc.Bacc("TRN2", target_bir_lowering=False, debug=False,
                   num_devices=N_CORES)

    # --- I/O ---
    xT_d = nc.dram_tensor("xT", [d_in, S], bf16, kind="ExternalInput").ap()
    deg_d = nc.dram_tensor("deg_w", [P, NW], f32, kind="ExternalInput").ap()
    idx_d = nc.dram_tensor("idx16", [P, TILES * P // 16], i16,
                           kind="ExternalInput").ap()
    oh_d = nc.dram_tensor("ohb", [P, TILES * P], bf16,
                          kind="ExternalInput").ap()
    batchl_d = nc.dram_tensor("batchl", [P, NW], f32, kind="ExternalInput").ap()
    W1_d = nc.dram_tensor("W1b", [d_in, h_dim], bf16, kind="ExternalInput").ap()
    W2_d = nc.dram_tensor("W2b", [h_dim, h_dim], bf16, kind="ExternalInput").ap()
    W3_d = nc.dram_tensor("W3b", [h_dim, h_dim], bf16, kind="ExternalInput").ap()
    Wf_d = nc.dram_tensor("Wf", [h_dim, n_cls], f32, kind="ExternalInput").ap()
    b_d = [nc.dram_tensor(f"b{i+1}b", [P, h_dim], f32, kind="ExternalInput").ap()
           for i in range(3)]
    bf_d = nc.dram_tensor("bfb", [P, n_cls], f32, kind="ExternalInput").ap()
    ident_d = nc.dram_tensor("identb", [P, P], bf16, kind="ExternalInput").ap()
    iota_g_d = nc.dram_tensor("iota_g", [P, GMAX], f32, kind="ExternalInput").ap()
    out_d = nc.dram_tensor("out", [GMAX, n_cls], f32, kind="ExternalOutput").ap()

    znloc_q = [nc.dram_tensor(f"znloc{q}", [SQ, P], bf16).ap()
               for q in range(NQ)]
    table_q = [nc.dram_tensor(f"table{q}", [QROWS, P], bf16,
                              addr_space="Shared").ap()
               for q in range(NQ)]
    rg = [list(range(N_CORES))]
    W_next = [W2_d, W3_d]

    with tile.TileContext(nc) as tc:
        with (
            tc.tile_pool(name="persist", bufs=1) as pp,
            tc.tile_pool(name="msg", bufs=4) as msgp,
            tc.tile_pool(name="ohst", bufs=3) as ohp,
            tc.tile_pool(name="dense", bufs=3) as dp,
            tc.tile_pool(name="work", bufs=4) as wp,
            tc.tile_pool(name="psA", bufs=2, space="PSUM") as psA,
            tc.tile_pool(name="psB", bufs=2, space="PSUM") as psB,
            tc.tile_pool(name="pool_ps", bufs=1, space="PSUM") as poolps,
        ):
            # --- persistent tiles ---
            ident = pp.tile([P, P], bf16, tag="ident")
            nc.sync.dma_start(ident[:], ident_d[:])
            iota_g = pp.tile([P, GMAX], f32, tag="iota_g")
            nc.sync.dma_start(iota_g[:], iota_g_d[:])
            W1 = pp.tile([d_in, h_dim], bf16, tag="W1")
            nc.sync.dma_start(W1[:], W1_d[:])
            W2 = pp.tile([h_dim, h_dim], bf16, tag="W2")
            nc.sync.dma_start(W2[:], W2_d[:])
            W3 = pp.tile([h_dim, h_dim], bf16, tag="W3")
            nc.sync.dma_start(W3[:], W3_d[:])
            Wn = [W2, W3]
            Wf_sb = pp.tile([h_dim, n_cls], f32, tag="Wf")
            nc.sync.dma_start(Wf_sb[:], Wf_d[:])
            b_sb = []
            for i in range(3):
                t = pp.tile([P, h_dim], f32, tag=f"b{i}")
                nc.sync.dma_start(t[:], b_d[i][:])
                b_sb.append(t)
            bf_sb = pp.tile([P, n_cls], f32, tag="bf")
            nc.sync.dma_start(bf_sb[:], bf_d[:])
            batchl_sb = pp.tile([P, NW], f32, tag="batchl")
            nc.sync.dma_start(batchl_sb[:], batchl_d[:])
            idx_sb = pp.tile([P, TILES * P // 16], i16, tag="idx")
            nc.sync.dma_start(idx_sb[:], idx_d[:])

            dinv = pp.tile([P, NW], f32, tag="dinv")
            deg_col = pp.tile([P, NW], f32, tag="degc")
            nc.sync.dma_start(deg_col[:], deg_d[:])
            nc.scalar.activation(deg_col[:], deg_col[:], AOT.Sqrt)
            nc.vector.reciprocal(dinv[:], deg_col[:])
            dinv2 = pp.tile([P, NW], f32, tag="dinv2")
            nc.vector.tensor_tensor(out=dinv2[:], in0=dinv[:], in1=dinv[:],
                                    op=ALU.mult)

            bufA = pp.tile([P, NW * h_dim], f32, tag="bufA")
            u_sb = pp.tile([P, NW * h_dim], f32, tag="u")
            znb_sb = pp.tile([P, NW * h_dim], bf16, tag="znb")
            h_sb = pp.tile([P, NW * h_dim], bf16, tag="h")

            def dense_publish(layer, w, lhsT):
                """z = lhsT^T @ W; znb = z*dinv (publish); u = z*dinv^2 + b.

                ACT copy-with-scale keeps this off the contended DVE port.
                """
                zps = psA.tile([P, h_dim], f32, tag="zps")
                Wt = W1 if layer == 0 else Wn[layer - 1]
                nc.tensor.matmul(zps[:], lhsT=lhsT, rhs=Wt[:],
                                 start=True, stop=True)
                nc.scalar.activation(
                    znb_sb[:, w * h_dim:(w + 1) * h_dim], zps[:], AOT.Copy,
                    scale=dinv[:, w:w + 1])
                uw = u_sb[:, w * h_dim:(w + 1) * h_dim]
                nc.scalar.activation(uw, zps[:], AOT.Copy,
                                     scale=dinv2[:, w:w + 1])
                nc.vector.tensor_tensor(out=uw, in0=uw, in1=b_sb[layer][:],
                                        op=ALU.add)

            deferred_ags = []

            def emit_ag(qq):
                nc.gpsimd.collective_compute(
                    "AllGather", ALU.bypass, replica_groups=rg,
                    ins=[znloc_q[qq][:]], outs=[table_q[qq][:]])

            def publish_quarter(layer, qq, defer=False):
                sl = znb_sb[:, qq * NWQ * h_dim:(qq + 1) * NWQ * h_dim]
                src = sl.rearrange("p (t d) -> p t d", d=h_dim)
                nc.sync.dma_start(
                    znloc_q[qq][:, 0:h_dim].rearrange("(t p) d -> p t d", p=P),
                    src)
                nc.sync.dma_start(
                    znloc_q[qq][:, h_dim:2 * h_dim].rearrange(
                        "(t p) d -> p t d", p=P),
                    src)
                if defer:
                    deferred_ags.append(qq)
                else:
                    emit_ag(qq)

            # === layer-0 dense from xT (streamed) + publish/AG all quarters ===
            for w in range(NW):
                xw = dp.tile([d_in, P], bf16, tag="xw")
                nc.sync.dma_start(xw[:], xT_d[:, w * P:(w + 1) * P])
                dense_publish(0, w, xw[:])
                if (w + 1) % NWQ == 0:
                    # only q0's AG blocks the first gathers; defer the rest
                    publish_quarter(0, w // NWQ, defer=(w // NWQ > 0))

            # === 3 GCN layers ===
            pool_started = [False]

            def flush_window(L, w, wps):
                """h_w = tanh(dinv*(msgsum) + u); wps = last pass PSUM or None."""
                uw = u_sb[:, w * h_dim:(w + 1) * h_dim]
                hw = h_sb[:, w * h_dim:(w + 1) * h_dim]
                if wps is None and first_pass[w] < 0:
                    nc.scalar.activation(hw, uw, AOT.Tanh)
                else:
                    acc = bufA[:, w * h_dim:(w + 1) * h_dim]
                    if first_pass[w] != last_pass[w]:
                        nc.vector.tensor_tensor(out=acc, in0=acc, in1=wps[:],
                                                op=ALU.add)
                        src = acc
                    else:
                        src = wps[:]
                    tmp = wp.tile([P, h_dim], f32, tag="ftmp")
                    nc.scalar.activation(tmp[:], src, AOT.Copy,
                                         scale=dinv[:, w:w + 1])
                    nc.vector.tensor_tensor(out=tmp[:], in0=tmp[:], in1=uw,
                                            op=ALU.add)
                    nc.scalar.activation(hw, tmp[:], AOT.Tanh)
                if L < 2:
                    # dense for next layer: transpose h_w then matmul
                    tp = psB.tile([h_dim, P], bf16, tag="tp")
                    nc.tensor.transpose(tp[:], hw, ident[:])
                    sbT = dp.tile([h_dim, P], bf16, tag="sbT")
                    nc.vector.tensor_copy(sbT[:], tp[:])
                    dense_publish(L + 1, w, sbT[:])
                else:
                    # pooling rides the last flush
                    ohg = wp.tile([P, GMAX], bf16, tag="ohg")
                    nc.vector.tensor_scalar(
                        out=ohg[:], in0=iota_g[:],
                        scalar1=batchl_sb[:, w:w + 1], scalar2=None,
                        op0=ALU.is_equal)
                    nc.tensor.matmul(poolT[:], lhsT=hw, rhs=ohg[:],
                                     start=not pool_started[0],
                                     stop=(flushed[0] == NW - 1))
                    pool_started[0] = True

            for L in range(3):
                wpsum = None
                flushed = [0]       # count of flushed windows (for pool stop)
                quarter_done = [0]  # publish progress for layer L+1
                win_flushed = np.zeros(NW, dtype=bool)

                def maybe_flush(w, wps):
                    flush_window(L, w, wps)
                    win_flushed[w] = True
                    flushed[0] += 1
                    if L < 2:
                        while quarter_done[0] < NQ and \
                                win_flushed[quarter_done[0] * NWQ:
                                            (quarter_done[0] + 1) * NWQ].all():
                            publish_quarter(L + 1, quarter_done[0],
                                            defer=(quarter_done[0] == NQ - 1))
                            quarter_done[0] += 1

                if L == 2:
                    poolT = poolps.tile([h_dim, GMAX], f32, tag="poolT")

                # windows with no edges at all: h = tanh(u) directly
                for w in range(NW):
                    if first_pass[w] < 0:
                        maybe_flush(w, None)

                for q in range(NQ):
                    # a deferred AG for this quarter must land before its pass
                    while deferred_ags and q in deferred_ags:
                        emit_ag(deferred_ags.pop(0))
                    t0, t1 = int(pass_t0[q]), int(pass_t0[q + 1])
                    gi = 0
                    for a in range(t0, t1, GRT):
                        if deferred_ags and gi in (5, 8, 11):
                            emit_ag(deferred_ags.pop(0))
                        gi += 1
                        gnt = min(GRT, t1 - a)
                        gsz = gnt * P
                        msg = msgp.tile([P, GRT * P], bf16, tag="msg")
                        nc.gpsimd.dma_gather(
                            msg[:, :gnt * P].rearrange("p (t d) -> p t d", d=P),
                            table_q[q],
                            idx_sb[:, a * P // 16:(a * P + gsz) // 16],
                            gsz, gsz, P, elem_step=P,
                            single_packet=False)
                        oht = ohp.tile([P, GRT * P], bf16, tag="oht")
                        nc.sync.dma_start(oht[:, :gnt * P],
                                          oh_d[:, a * P:a * P + gsz])
                        for i in range(gnt):
                            t = a + i
                            w = int(tile_win[t])
                            if tile_first[t]:
                                wpsum = psA.tile([P, h_dim], f32, tag="wps")
                            nc.tensor.matmul(
                                wpsum[:],
                                lhsT=oht[:, i * P:(i + 1) * P],
                                rhs=msg[:].rearrange(
                                    "p (t d) -> p t d", d=P)[:, i, 0:h_dim],
                                start=bool(tile_first[t]),
                                stop=bool(tile_last[t]))
                            if tile_last[t]:
                                if last_pass[w] == q:
                                    maybe_flush(w, wpsum)
                                else:
                                    dst = bufA[:, w * h_dim:(w + 1) * h_dim]
                                    if first_pass[w] == q:
                                        nc.vector.tensor_copy(dst, wpsum[:])
                                    else:
                                        nc.vector.tensor_tensor(
                                            out=dst, in0=dst, in1=wpsum[:],
                                            op=ALU.add)
                assert flushed[0] == NW, (L, flushed[0])
            assert not deferred_ags, deferred_ags

            # === head: out = tanh(pooled @ Wf + bf) ===
            poolS = pp.tile([h_dim, GMAX], f32, tag="poolS")
            nc.vector.tensor_copy(poolS[:], poolT[:])
            for gt in range(GMAX // P):
                fps = psA.tile([P, n_cls], f32, tag="zps")
                nc.tensor.matmul(fps[:], lhsT=poolS[:, gt * P:(gt + 1) * P],
                                 rhs=Wf_sb[:], start=True, stop=True)
                ot = wp.tile([P, n_cls], f32, tag="ot")
                nc.vector.tensor_tensor(out=ot[:], in0=fps[:], in1=bf_sb[:],
                                        op=ALU.add)
                nc.scalar.activation(ot[:], ot[:], AOT.Tanh)
                nc.sync.dma_start(out_d[gt * P:(gt + 1) * P, :], ot[:])

    nc.compile()
    return nc


# ----------------------------------------------------------------------------
# Runner (persistent compiled program + per-core inputs)
# ----------------------------------------------------------------------------

_DYNAMIC_INPUTS = ("xT", "W1b", "W2b", "W3b", "Wf", "b1b", "b2b", "b3b", "bfb")


class Runner:
    def __init__(self, meta, nc, d_in, h_dim, n_cls):
        self.meta = meta
        self.nc = nc
        self.d_in, self.h_dim, self.n_cls = d_in, h_dim, n_cls
        self._timed = None

    def _ensure_timed(self, maps):
        """Build (once) a jitted runner with device-resident inputs; re-upload
        only the dynamic tensors (x, weights, biases) on later calls."""
        import jax
        import concourse.mybir as mybir
        from concourse import bass2jax
        from jax.experimental.shard_map import shard_map
        from jax.sharding import Mesh, NamedSharding, PartitionSpec

        nc = self.nc
        if self._timed is None:
            bass2jax.install_neuronx_cc_hook()
            partition_name = (nc.partition_id_tensor.name
                              if nc.partition_id_tensor else None)
            in_names, out_names, out_avals, zero_shapes = [], [], [], []
            for alloc in nc.m.functions[0].allocations:
                if not isinstance(alloc, mybir.MemoryLocationSet):
                    continue
                name = alloc.memorylocations[0].name
                if alloc.kind == "ExternalInput":
                    if name != partition_name:
                        in_names.append(name)
                elif alloc.kind == "ExternalOutput":
                    shape = tuple(alloc.tensor_shape)
                    dtype = mybir.dt.np(alloc.dtype)
                    out_names.append(name)
                    out_avals.append(jax.core.ShapedArray(shape, dtype))
                    zero_shapes.append(((N_CORES * shape[0],) + shape[1:], dtype))
            n_params = len(in_names)
            all_in = list(in_names) + list(out_names)
            if partition_name is not None:
                all_in.append(partition_name)
            donate = tuple(range(n_params, n_params + len(out_names)))

            def _body(*args):
                operands = list(args)
                if partition_name is not None:
                    operands.append(bass2jax.partition_id_tensor())
                return tuple(bass2jax._bass_exec_p.bind(
                    *operands, out_avals=tuple(out_avals),
                    in_names=tuple(all_in), out_names=tuple(out_names),
                    lowering_input_output_aliases=(),
                    sim_require_finite=True, sim_require_nnan=True, nc=nc))

            devices = jax.devices()[:N_CORES]
            mesh = Mesh(np.asarray(devices), ("core",))
            spec = NamedSharding(mesh, PartitionSpec("core"))
            fn = jax.jit(
                shard_map(_body, mesh=mesh,
                          in_specs=(PartitionSpec("core"),) * (n_params + len(out_names)),
                          out_specs=(PartitionSpec("core"),) * len(out_names)),
                donate_argnums=donate, keep_unused=True)
            self._timed = dict(fn=fn, in_names=in_names, out_names=out_names,
                               out_avals=out_avals, zero_shapes=zero_shapes,
                               spec=spec, dev={})
        st = self._timed
        for nm in st["in_names"]:
            if nm in st["dev"] and nm not in _DYNAMIC_INPUTS:
                continue
            arr = np.concatenate([np.asarray(maps[c][nm])
                                  for c in range(N_CORES)], axis=0)
            st["dev"][nm] = jax.device_put(arr, st["spec"])
        return st

    def run_fast(self, maps):
        import jax
        st = self._ensure_timed(maps)
        zeros = [jax.device_put(np.zeros(s, d), st["spec"])
                 for s, d in st["zero_shapes"]]
        outs = st["fn"](*[st["dev"][nm] for nm in st["in_names"]], *zeros)
        outs = [np.asarray(o) for o in outs]
        results = []
        for c in range(N_CORES):
            results.append({nm: outs[i].reshape(N_CORES, *st["out_avals"][i].shape)[c]
                            for i, nm in enumerate(st["out_names"])})
        return self.assemble(results)

    def in_maps(self, x, W1, b1, W2, b2, W3, b3, Wf, bf):
        import ml_dtypes
        bf16 = ml_dtypes.bfloat16
        m = self.meta
        S = m["S"]
        C = N_CORES
        x = np.asarray(x, np.float32)
        maps = []
        reps = dict(
            W1b=np.asarray(W1, np.float32).astype(bf16),
            W2b=np.asarray(W2, np.float32).astype(bf16),
            W3b=np.asarray(W3, np.float32).astype(bf16),
            Wf=np.asarray(Wf, np.float32),
            b1b=np.broadcast_to(np.asarray(b1, np.float32), (P, self.h_dim)).copy(),
            b2b=np.broadcast_to(np.asarray(b2, np.float32), (P, self.h_dim)).copy(),
            b3b=np.broadcast_to(np.asarray(b3, np.float32), (P, self.h_dim)).copy(),
            bfb=np.broadcast_to(np.asarray(bf, np.float32), (P, self.n_cls)).copy(),
            identb=np.eye(P, dtype=np.float32).astype(bf16),
            iota_g=np.broadcast_to(np.arange(GMAX, dtype=np.float32),
                                   (P, GMAX)).copy(),
        )
        for c in range(C):
            n0, n1 = int(m["node_bnds"][c]), int(m["node_bnds"][c + 1])
            xl = np.zeros((S, self.d_in), np.float32)
            mask = m["perm"][c] >= 0
            xl[mask] = x[n0:n1][m["perm"][c][mask]]
            maps.append(dict(
                xT=np.ascontiguousarray(xl.T).astype(bf16),
                deg_w=m["deg_w"][c],
                idx16=np.tile(m["idx_wrapped"][c], (8, 1)),
                ohb=m["ohb"][c],
                batchl=m["batchl_w"][c],
                **reps,
            ))
        return maps

    def run(self, maps):
        from concourse.bass_utils import run_bass_kernel_spmd
        res = run_bass_kernel_spmd(self.nc, maps, list(range(N_CORES)))
        return self.assemble(res.results)

    def assemble(self, results):
        m = self.meta
        outs = []
        for c in range(N_CORES):
            outs.append(results[c]["out"][: int(m["g_cnt"][c])])
        return np.concatenate(outs, axis=0)


_CACHE = {}


def _get_runner(edge_index, batch, n_nodes, n_graphs, d_in, h_dim, n_cls):
    key = (hashlib.sha1(np.ascontiguousarray(edge_index).tobytes()).hexdigest(),
           hashlib.sha1(np.ascontiguousarray(batch).tobytes()).hexdigest(),
           n_nodes, n_graphs, d_in, h_dim, n_cls)
    r = _CACHE.get(key)
    if r is None:
        meta = _preprocess(edge_index, batch, n_nodes, n_graphs)
        nc = _build_program(meta, d_in, h_dim, n_cls)
        r = Runner(meta, nc, d_in, h_dim, n_cls)
        _CACHE[key] = r
    return r


def kernel(x, edge_index, batch, W1, b1, W2, b2, W3, b3, Wf, bf):
    x = np.asarray(x)
    r = _get_runner(np.asarray(edge_index), np.asarray(batch), x.shape[0],
                    N_GRAPHS, x.shape[1], np.asarray(W1).shape[1],
                    np.asarray(Wf).shape[1])
    maps = r.in_maps(x, W1, b1, W2, b2, W3, b3, Wf, bf)
    return r.run_fast(maps)
